# revision 1
# baseline (speedup 1.0000x reference)
"""YOLO-style loss kernel for Trainium2, data-parallel over 8 NeuronCores.

v4: fp16 feature-major layout; DMA-accumulated cls diffs; grouped
square+accumulate; engine-balanced; early wh DMA for the IoU chain.

Host prep is layout-only (transpose, fp16 cast, constant scales: xy rows
by k=2/S, cls target rows by -1); all data-dependent math is on device.

Per core: X[128, 60, 784] fp16 rows:
  0:4    p_wh  [w0,h0,w1,h1]
  4:8    t_wh  (raw)
  8:14   p_xyc [x0,y0,c0,x1,y1,c1], xy rows scaled by k
  14:20  t_xyc (same scaling, raw sign)
  then per cls tile (8,6,4,2 rows): p_cls rows, then (-t_cls) rows.

Device: cls diff arrives via accumulate-DMA (p + (-t)) done by the DMA
engines (gpsimd software DGE), so DVE never subtracts cls data.

Math (scale-free responsible-box form):
  resp0 = [i0*s1 >= i1*s0], i_j = relu((pw+tw)-max(|k dxy|,|dwh|))_w*(.)_h,
  s_j = pw*ph + tw*th.
  obj*F_j^2 = m_j*D_j^2 + mo_j*p_j^2, m_j = obj*resp_j, mo = m swapped.
  noobj conf = 0.5*(sum_all dconf^2 - sum_obj dconf^2).

acc[P,16] fp32 group sums (host weights + partition sum):
  0 xy-D  1 xy-P  2 c(D+P)  4 corr(-0.5)  5 raw(+0.5)  6..9 cls tiles
"""

import numpy as np

N_CORES = 8
BATCH = 16384
S = 7
P = 128
SHARD = BATCH // N_CORES
CELLS = SHARD * S * S
CPP = CELLS // P                  # 784
K_XY = 2.0 / S

LAMBDA_COORD = 5.0
LAMBDA_NOOBJ = 0.5

_CACHE = {}

CLS_SPLIT = [8, 6, 4, 2]


def _split_waits(nc, max_waits=1):
    import concourse.mybir as mybir

    n_new = 0
    for fn in nc.m.functions:
        for blk in fn.blocks:
            out = []
            changed = False
            for inst in list(blk.instructions):
                si = inst.sync_info
                ow = list(si.on_wait) if si is not None else []
                if len(ow) > max_waits:
                    for w in ow[:-max_waits]:
                        n_new += 1
                        out.append(
                            mybir.InstNoOp(
                                name=f"waitnop-{n_new}-{inst.name}",
                                engine=inst.engine,
                                ins=[],
                                outs=[],
                                sync_info=mybir.SyncInfo(
                                    on_wait=[w], on_update=[]
                                ),
                            )
                        )
                    inst.sync_info = mybir.SyncInfo(
                        on_wait=ow[-max_waits:], on_update=list(si.on_update)
                    )
                    changed = True
                out.append(inst)
            if changed:
                blk.instructions = out
    return n_new


def _build_bass(reps=1):
    import concourse.bass as bass
    import concourse.mybir as mybir
    from contextlib import ExitStack
    from concourse.tile import TileContext

    f16 = mybir.dt.float16
    f32 = mybir.dt.float32
    Op = mybir.AluOpType
    Act = mybir.ActivationFunctionType
    C = CPP

    nc = bass.Bass()
    xin = nc.dram_tensor("xin", [P, 60, C], f16, kind="ExternalInput")
    out = nc.dram_tensor("out", [P, 16], f32, kind="ExternalOutput")

    cls_rows = []
    r0 = 20
    for nd in CLS_SPLIT:
        cls_rows.append((r0, nd))
        r0 += 2 * nd

    with ExitStack() as ctx:
        tc = ctx.enter_context(TileContext(nc))
        xb = ctx.enter_context(tc.tile_pool(name="xb", bufs=1))
        xc = ctx.enter_context(tc.tile_pool(name="xc", bufs=1))
        work = ctx.enter_context(tc.tile_pool(name="work", bufs=1))
        singles = ctx.enter_context(tc.tile_pool(name="singles", bufs=1))

        acc = singles.tile([P, 16], f32, tag="acc")
        zeros = singles.tile([P, 1], f16, tag="zeros")
        nc.vector.memset(zeros, 0.0)

        for rep in range(reps):
            # ---- DMAs: wh first (starts IoU chain early), then xyc,
            # then per cls tile: p rows + accumulate (-t) rows onto them
            xwh = xb.tile([P, 8, C], f16, tag="xwh")
            nc.sync.dma_start(out=xwh, in_=xin[:, 0:8])
            xxy = xb.tile([P, 8, C], f16, tag="xxy")
            nc.sync.dma_start(out=xxy, in_=xin[:, 8:16])
            xcf = xb.tile([P, 4, C], f16, tag="xcf")
            nc.sync.dma_start(out=xcf, in_=xin[:, 16:20])
            xcls = []
            for kk, (r0, nd) in enumerate(cls_rows):
                xk = xc.tile([P, 2 * nd, C], f16, tag=f"xc{kk}")
                nc.sync.dma_start(out=xk, in_=xin[:, r0 : r0 + 2 * nd])
                xcls.append(xk)

            Pwh = xwh[:, 0:4].rearrange("p (b g) c -> p b g c", b=2)
            Twh = xwh[:, 4:8].rearrange("p (b g) c -> p b g c", b=2)
            whv = xwh.rearrange("p (t r) c -> p t r c", t=2)
            wv = whv[:, :, 0:4:2]
            hv = whv[:, :, 1:4:2]
            Pxy4 = xxy[:, 0:4]
            Txy4 = xxy[:, 4:8]
            Pc = xcf[:, 0:2]
            Tc = xcf[:, 2:4]

            # ---- early DVE work on wh tile ----
            dwh = work.tile([P, 4, C], f16, tag="dwh")
            nc.vector.tensor_tensor(
                out=dwh, in0=xwh[:, 0:4], in1=xwh[:, 4:8], op=Op.subtract
            )
            Dwh = dwh.rearrange("p (b g) c -> p b g c", b=2)
            sums = work.tile([P, 2, 2, C], f16, tag="sums")
            nc.vector.tensor_tensor(out=sums, in0=Pwh, in1=Twh, op=Op.add)
            ar = work.tile([P, 2, 2, C], f16, tag="ar")
            nc.gpsimd.tensor_tensor(out=ar, in0=wv, in1=hv, op=Op.mult)
            s = work.tile([P, 2, C], f16, tag="s")
            nc.gpsimd.tensor_tensor(
                out=s, in0=ar[:, 0], in1=ar[:, 1], op=Op.add
            )

            # ---- xyc diffs + IoU chain ----
            D6 = work.tile([P, 6, C], f16, tag="D6")
            nc.vector.tensor_tensor(
                out=D6[:, 0:4], in0=Pxy4, in1=Txy4, op=Op.subtract
            )
            nc.vector.tensor_tensor(
                out=D6[:, 4:6], in0=Pc, in1=Tc, op=Op.subtract
            )
            Dxy = D6[:, 0:4].rearrange("p (b g) c -> p b g c", b=2)
            Dc = D6[:, 4:6]
            # abs via (t-p) then max with (p-t): contiguous 2x ops
            negwh = work.tile([P, 4, C], f16, tag="negwh")
            nc.gpsimd.tensor_tensor(
                out=negwh, in0=xwh[:, 4:8], in1=xwh[:, 0:4], op=Op.subtract
            )
            awh = work.tile([P, 2, 2, C], f16, tag="awh")
            nc.vector.tensor_tensor(
                out=awh.rearrange("p a b c -> p (a b c)"),
                in0=dwh.rearrange("p r c -> p (r c)"),
                in1=negwh.rearrange("p r c -> p (r c)"), op=Op.max,
            )
            negxy = work.tile([P, 4, C], f16, tag="negxy")
            nc.vector.tensor_tensor(
                out=negxy, in0=Txy4, in1=Pxy4,
                op=Op.subtract,
            )
            axy = work.tile([P, 2, 2, C], f16, tag="axy")
            nc.vector.tensor_tensor(
                out=axy.rearrange("p a b c -> p (a b c)"),
                in0=D6[:, 0:4].rearrange("p r c -> p (r c)"),
                in1=negxy.rearrange("p r c -> p (r c)"), op=Op.max,
            )
            mx = work.tile([P, 2, 2, C], f16, tag="mx")
            nc.vector.tensor_tensor(
                out=mx.rearrange("p a b c -> p (a b c)"),
                in0=axy.rearrange("p a b c -> p (a b c)"),
                in1=awh.rearrange("p a b c -> p (a b c)"), op=Op.max,
            )
            iwv = work.tile([P, 2, 2, C], f16, tag="iwv")
            nc.vector.tensor_tensor(
                out=iwv.rearrange("p a b c -> p (a b c)"),
                in0=sums.rearrange("p a b c -> p (a b c)"),
                in1=mx.rearrange("p a b c -> p (a b c)"), op=Op.subtract,
            )
            r2 = work.tile([P, 2, 2, C], f16, tag="r2")
            nc.vector.tensor_tensor(
                out=r2.rearrange("p a b c -> p (a b c)"),
                in0=iwv.rearrange("p a b c -> p (a b c)"),
                in1=zeros.broadcast_to([P, 4 * C]), op=Op.max,
            )
            inter = work.tile([P, 2, C], f16, tag="inter")
            nc.vector.tensor_tensor(
                out=inter, in0=r2[:, :, 0], in1=r2[:, :, 1], op=Op.mult
            )
            lr = work.tile([P, 2, C], f16, tag="lr")
            nc.vector.tensor_tensor(
                out=lr[:, 0], in0=inter[:, 0], in1=s[:, 1], op=Op.mult
            )
            nc.vector.tensor_tensor(
                out=lr[:, 1], in0=inter[:, 1], in1=s[:, 0], op=Op.mult
            )
            rr = work.tile([P, 2, C], f16, tag="rr")
            nc.vector.tensor_tensor(
                out=rr[:, 0], in0=lr[:, 0], in1=lr[:, 1], op=Op.is_ge
            )
            nc.vector.tensor_tensor(
                out=rr[:, 1], in0=lr[:, 0], in1=lr[:, 1], op=Op.is_lt
            )

            # obj on Pool (off the DVE chain), corr product after it
            obj = singles.tile([P, C], f16, tag="obj")
            nc.gpsimd.tensor_scalar(
                out=obj, in0=Tc[:, 0], scalar1=0.0, scalar2=None,
                op0=Op.is_gt,
            )
            gc2 = work.tile([P, 2, C], f16, tag="gc2")
            nc.gpsimd.tensor_tensor(
                out=gc2, in0=Dc,
                in1=obj.unsqueeze(1).broadcast_to([P, 2, C]), op=Op.mult,
            )

            m = work.tile([P, 2, C], f16, tag="m")
            nc.vector.tensor_tensor(
                out=m, in0=rr,
                in1=obj.unsqueeze(1).broadcast_to([P, 2, C]), op=Op.mult,
            )
            mo = m[:, ::-1]

            # ---- cls tiles: mask (DVE); squares: tiles 1-2 on DVE
            # (mult + flat reduce), tiles 3-4 on ACT (accum_out) ----
            for kk, (r0, nd) in enumerate(cls_rows):
                xk = xcls[kk]
                dk = xk[:, 0:nd]
                nc.vector.tensor_tensor(
                    out=dk, in0=xk[:, 0:nd], in1=xk[:, nd : 2 * nd],
                    op=Op.subtract,
                )
                nc.vector.tensor_tensor(
                    out=dk, in0=dk,
                    in1=obj.unsqueeze(1).broadcast_to([P, nd, C]),
                    op=Op.mult,
                )
                nc.scalar.activation(
                    out=dk, in_=dk, func=Act.Square,
                    accum_out=acc[:, 6 + kk : 7 + kk],
                )

            # ---- grouped masked products (DVE) + squares (ACT) ----
            braw = work.tile([P, 2, C], f16, tag="braw")
            nc.scalar.activation(
                out=braw, in_=Dc, func=Act.Square, accum_out=acc[:, 5:6]
            )
            braw2 = work.tile([P, 2, C], f16, tag="braw2")
            nc.scalar.activation(
                out=braw2, in_=gc2, func=Act.Square, accum_out=acc[:, 4:5]
            )
            # xy products (D-part and P-part share weight) in one tile,
            # one ACT square+accum -> col 0
            gmp = work.tile([P, 2, 2, 2, C], f16, tag="gmp")
            nc.vector.tensor_tensor(
                out=gmp[:, 0], in0=Dxy,
                in1=m.unsqueeze(2).broadcast_to([P, 2, 2, C]), op=Op.mult,
            )
            Pxy = Pxy4.rearrange("p (b g) c -> p b g c", b=2)
            nc.vector.tensor_tensor(
                out=gmp[:, 1], in0=Pxy,
                in1=mo.unsqueeze(2).broadcast_to([P, 2, 2, C]), op=Op.mult,
            )
            nc.scalar.activation(
                out=gmp, in_=gmp, func=Act.Square, accum_out=acc[:, 0:1]
            )
            # c products squared+reduced on DVE -> col 2
            gc = work.tile([P, 2, 2, C], f16, tag="gc")
            nc.vector.tensor_tensor(out=gc[:, 0], in0=Dc, in1=m, op=Op.mult)

            nc.vector.tensor_tensor(out=gc[:, 1], in0=Pc, in1=mo, op=Op.mult)
            nc.vector.tensor_tensor(
                out=gc.rearrange("p a b c -> p (a b c)"),
                in0=gc.rearrange("p a b c -> p (a b c)"),
                in1=gc.rearrange("p a b c -> p (a b c)"), op=Op.mult,
            )
            nc.vector.tensor_reduce(
                out=acc[:, 2:3], in_=gc.rearrange("p a b c -> p (a b c)"),
                axis=mybir.AxisListType.X, op=Op.add,
            )

        nc.vector.memset(acc[:, 1:2], 0.0)
        nc.vector.memset(acc[:, 3:4], 0.0)
        nc.vector.memset(acc[:, 10:16], 0.0)
        nc.sync.dma_start(out=out[:, :], in_=acc)

    _split_waits(nc)
    return nc


def _get_nc():
    if "nc" not in _CACHE:
        _CACHE["nc"] = _build_bass()
    return _CACHE["nc"]


def _weights():
    w = np.zeros(16, dtype=np.float64)
    wxy = LAMBDA_COORD / (K_XY * K_XY)
    w[0] = w[1] = wxy
    w[2] = 1.0
    w[4] = -LAMBDA_NOOBJ
    w[5] = LAMBDA_NOOBJ
    w[6:10] = 1.0
    return w


def _prep_shards(pred, targ):
    p = np.asarray(pred, dtype=np.float32).reshape(N_CORES, P, CPP, 30)
    t = np.asarray(targ, dtype=np.float32).reshape(N_CORES, P, CPP, 30)
    X = np.empty((N_CORES, P, 60, CPP), dtype=np.float16)
    wh_order = [2, 3, 7, 8]               # w0,h0,w1,h1
    xy_order = [0, 1, 5, 6]               # x0,y0,x1,y1
    c_order = [4, 9]                      # c0,c1
    X[:, :, 0:4] = p[..., wh_order].transpose(0, 1, 3, 2)
    X[:, :, 4:8] = t[..., wh_order].transpose(0, 1, 3, 2)
    X[:, :, 8:12] = p[..., xy_order].transpose(0, 1, 3, 2) * K_XY
    X[:, :, 12:16] = t[..., xy_order].transpose(0, 1, 3, 2) * K_XY
    X[:, :, 16:18] = p[..., c_order].transpose(0, 1, 3, 2)
    X[:, :, 18:20] = t[..., c_order].transpose(0, 1, 3, 2)
    r0, lo = 20, 10
    for nd in CLS_SPLIT:
        X[:, :, r0 : r0 + nd] = p[..., lo : lo + nd].transpose(0, 1, 3, 2)
        X[:, :, r0 + nd : r0 + 2 * nd] = t[..., lo : lo + nd].transpose(
            0, 1, 3, 2
        )
        r0 += 2 * nd
        lo += nd
    return [np.ascontiguousarray(X[c]) for c in range(N_CORES)]


def _host_combine(outs):
    w = _weights()
    total = 0.0
    for o in outs:
        per_f = np.asarray(o, dtype=np.float64).reshape(P, 16).sum(axis=0)
        total += float(per_f @ w)
    return np.float32(total / BATCH)


def _run(inputs, trace=False):
    from concourse.bass_utils import run_bass_kernel_spmd

    shards = _prep_shards(inputs["predictions"], inputs["targets"])
    in_maps = [{"xin": shards[c]} for c in range(N_CORES)]
    res = run_bass_kernel_spmd(
        _get_nc(), in_maps, core_ids=list(range(N_CORES)), trace=trace
    )
    loss = _host_combine([r["out"] for r in res.results])
    return loss, res


def kernel(predictions, targets):
    loss, _ = _run({"predictions": predictions, "targets": targets})
    return loss

